# revision 3
# baseline (speedup 1.0000x reference)
"""GCNII conv kernel for 8 Trainium2 NeuronCores.

Strategy (self-contained; shapes hardcoded):
  - Shard destination nodes across 8 cores (6250 each); edges partitioned by
    destination so each core's segment_sum is local.
  - Host pre-pass: sort edges by dest, group into 128-dest tiles, split each
    tile's edges by source half (int16 gather indices), pad each half to a
    multiple of 128 ("chunks"); per-tile chunk counts are the max over cores
    so all cores run one identical program.
  - Device, per dest tile:
      * dma_gather pulls all the tile's source rows x[row] (512B each) into
        SBUF as [128 edges, chunk, 128 feat] (idx i -> dst[i%128, i//128, :])
      * per chunk, one fused DVE op builds the scaled scatter matrix
        S[e, d] = 0.9*norm[e] * (col_local[e] == d)   (iota==col, then *norm)
      * PE accumulates segT[f, d] += msgs[e, f].T @ S[e, d] in PSUM
      * hT = segT + (0.1*x0).T tile  (alpha folded on host)
      * yT = W_eff @ hT via one matmul, W_eff = (1-beta)*I + beta*W folded on
        host, so no extra elementwise work
  - Output is produced transposed ([128, n_local]) and flipped back on host.
"""

import os
import sys

sys.path.insert(0, "/opt/trn_rl_repo")

import numpy as np

N = 50000
D = 128
NCORES = 8
NPC = N // NCORES          # 6250 dest nodes per core
TPC = (NPC + 127) // 128   # 49 dest tiles per core
NPAD = TPC * 128           # 6272
HALF = N // 2              # int16 gather index split
ALPHA = 0.1
THETA = 0.5
LAYER = 1

_prog_cache = {}

# Stash of the last BassKernelResults for test.py to inspect (exec_time_ns).
LAST = None


def _build_program(schedule):
    """schedule: list of (Mlo, Mhi) per dest tile (shared across cores)."""
    import concourse.bacc as bacc
    import concourse.mybir as mybir
    import concourse.tile as tile
    from concourse import library_config

    f32 = mybir.dt.float32
    i16 = mybir.dt.int16
    TC = sum(ml + mh for ml, mh in schedule)
    CLO8 = sum(ml for ml, _ in schedule) * 8
    CHI8 = sum(mh for _, mh in schedule) * 8

    nc = bacc.Bacc(
        "TRN2", target_bir_lowering=False, debug=False, num_devices=NCORES
    )
    xlo = nc.dram_tensor("xlo", [HALF, D], f32, kind="ExternalInput").ap()
    xhi = nc.dram_tensor("xhi", [N - HALF, D], f32, kind="ExternalInput").ap()
    ilo = nc.dram_tensor("ilo", [128, CLO8], i16, kind="ExternalInput").ap()
    ihi = nc.dram_tensor("ihi", [128, CHI8], i16, kind="ExternalInput").ap()
    cols = nc.dram_tensor("cols", [128, TC], f32, kind="ExternalInput").ap()
    nrm = nc.dram_tensor("nrm", [128, TC], f32, kind="ExternalInput").ap()
    iot = nc.dram_tensor("iot", [128, 128], f32, kind="ExternalInput").ap()
    x0t = nc.dram_tensor("x0t", [D, NPAD], f32, kind="ExternalInput").ap()
    wl = nc.dram_tensor("wl", [D, D], f32, kind="ExternalInput").ap()
    yt = nc.dram_tensor("yt", [D, NPAD], f32, kind="ExternalOutput").ap()

    with tile.TileContext(nc) as tc:
        with (
            tc.tile_pool(name="persist", bufs=1) as pp,
            tc.tile_pool(name="msgs", bufs=2) as mp,
            tc.tile_pool(name="sel", bufs=4) as sp,
            tc.tile_pool(name="hout", bufs=2) as hp,
            tc.tile_pool(name="io", bufs=2) as iop,
            tc.tile_pool(name="pseg", bufs=2, space="PSUM") as psp,
            tc.tile_pool(name="py", bufs=2, space="PSUM") as pyp,
        ):
            nc.gpsimd.load_library(library_config.mlp)

            ilo_sb = pp.tile([128, CLO8], i16)
            ihi_sb = pp.tile([128, CHI8], i16)
            cols_sb = pp.tile([128, TC], f32)
            nrm_sb = pp.tile([128, TC], f32)
            wl_sb = pp.tile([128, 128], f32)
            iota_f = pp.tile([128, 128], f32)

            nc.sync.dma_start(ilo_sb[:], ilo[:, :])
            nc.sync.dma_start(ihi_sb[:], ihi[:, :])
            nc.sync.dma_start(cols_sb[:], cols[:, :])
            nc.sync.dma_start(nrm_sb[:], nrm[:, :])
            nc.sync.dma_start(wl_sb[:], wl[:, :])
            nc.sync.dma_start(iota_f[:], iot[:, :])

            ci = 0
            lo_off = 0
            hi_off = 0
            for t, (Mlo, Mhi) in enumerate(schedule):
                M = Mlo + Mhi
                msgs = mp.tile([128, M, 128], f32, tag="msgs")
                if Mlo:
                    nc.gpsimd.dma_gather(
                        msgs[:, 0:Mlo, :],
                        xlo[:, :],
                        ilo_sb[:, lo_off * 8 : (lo_off + Mlo) * 8],
                        Mlo * 128,
                        Mlo * 128,
                        D,
                    )
                if Mhi:
                    nc.gpsimd.dma_gather(
                        msgs[:, Mlo:M, :],
                        xhi[:, :],
                        ihi_sb[:, hi_off * 8 : (hi_off + Mhi) * 8],
                        Mhi * 128,
                        Mhi * 128,
                        D,
                    )
                ps = psp.tile([128, 128], f32, space="PSUM", tag="pseg")
                for j in range(M):
                    S = sp.tile([128, 128], f32, tag="sel")
                    nc.vector.tensor_scalar(
                        out=S[:],
                        in0=iota_f[:],
                        scalar1=cols_sb[:, ci + j : ci + j + 1],
                        scalar2=nrm_sb[:, ci + j : ci + j + 1],
                        op0=mybir.AluOpType.is_equal,
                        op1=mybir.AluOpType.mult,
                    )
                    nc.tensor.matmul(
                        ps[:],
                        lhsT=msgs[:, j, :],
                        rhs=S[:],
                        start=(j == 0),
                        stop=(j == M - 1),
                    )
                x0tile = iop.tile([128, 128], f32, tag="x0")
                nc.sync.dma_start(x0tile[:], x0t[:, t * 128 : (t + 1) * 128])
                hT = hp.tile([128, 128], f32, tag="h")
                nc.vector.tensor_tensor(
                    out=hT[:], in0=ps[:], in1=x0tile[:], op=mybir.AluOpType.add
                )
                yp = pyp.tile([128, 128], f32, space="PSUM", tag="py")
                nc.tensor.matmul(
                    yp[:], lhsT=wl_sb[:], rhs=hT[:], start=True, stop=True
                )
                yo = iop.tile([128, 128], f32, tag="yo")
                nc.vector.tensor_copy(yo[:], yp[:])
                nc.sync.dma_start(yt[:, t * 128 : (t + 1) * 128], yo[:])
                ci += M
                lo_off += Mlo
                hi_off += Mhi

    nc.compile()
    return nc


def _wrap16(idx_list):
    """int16 idx list (len = M*128) -> [128, M*8] wrapped+replicated layout:
    idx i is read from partition i%16, free slot i//16; replicate x8."""
    w = idx_list.reshape(-1, 16).T.astype(np.int16)  # [16, M*8]
    return np.tile(w, (8, 1))


def _preprocess(x, x0, edge_index, norm, W):
    row = np.ascontiguousarray(edge_index[0]).astype(np.int64)
    col = np.ascontiguousarray(edge_index[1]).astype(np.int64)
    norm = np.ascontiguousarray(norm).astype(np.float32)
    x = np.ascontiguousarray(x).astype(np.float32)
    x0 = np.ascontiguousarray(x0).astype(np.float32)
    W = np.ascontiguousarray(W).astype(np.float32)

    order = np.argsort(col, kind="stable")
    rs = row[order]
    cs = col[order]
    ns = (1.0 - ALPHA) * norm[order]

    bounds = np.empty((NCORES, TPC + 1), dtype=np.int64)
    for c in range(NCORES):
        lo = c * NPC
        cuts = lo + np.minimum(np.arange(TPC + 1) * 128, NPC)
        bounds[c] = np.searchsorted(cs, cuts, side="left")

    # Per (core, tile): split by source half, count chunks per half.
    per_ct = {}
    Mlo_ct = np.zeros((NCORES, TPC), dtype=np.int64)
    Mhi_ct = np.zeros((NCORES, TPC), dtype=np.int64)
    for c in range(NCORES):
        for t in range(TPC):
            e0, e1 = bounds[c, t], bounds[c, t + 1]
            r = rs[e0:e1]
            cl = (cs[e0:e1] - (c * NPC + t * 128)).astype(np.float32)
            nn = ns[e0:e1]
            m = r < HALF
            per_ct[(c, t)] = (r[m], cl[m], nn[m], r[~m] - HALF, cl[~m], nn[~m])
            Mlo_ct[c, t] = -(-len(r[m]) // 128)
            Mhi_ct[c, t] = -(-(len(r) - len(r[m])) // 128)

    Mlo_t = Mlo_ct.max(axis=0)
    Mhi_t = Mhi_ct.max(axis=0)
    # Guarantee at least one chunk per tile so PSUM is always written.
    zero = (Mlo_t + Mhi_t) == 0
    Mlo_t[zero] = 1
    schedule = [(int(a), int(b)) for a, b in zip(Mlo_t, Mhi_t)]
    TC = int((Mlo_t + Mhi_t).sum())
    CLO = int(Mlo_t.sum())
    CHI = int(Mhi_t.sum())

    beta = np.float32(np.log(THETA / LAYER + 1.0))
    W_eff = (1.0 - beta) * np.eye(D, dtype=np.float32) + beta * W
    wl = np.ascontiguousarray(W_eff.T)
    iot = np.ascontiguousarray(
        np.tile(np.arange(128, dtype=np.float32)[None, :], (128, 1))
    )
    xlo = np.ascontiguousarray(x[:HALF])
    xhi = np.ascontiguousarray(x[HALF:])

    in_maps = []
    for c in range(NCORES):
        ilo_a = np.zeros((128, CLO * 8), dtype=np.int16)
        ihi_a = np.zeros((128, CHI * 8), dtype=np.int16)
        cols_a = np.full((128, TC), -1.0, dtype=np.float32)
        nrm_a = np.zeros((128, TC), dtype=np.float32)
        ci = 0
        lo_off = 0
        hi_off = 0
        for t in range(TPC):
            rl, cll, nl, rh, clh, nh = per_ct[(c, t)]
            Mlo, Mhi = int(Mlo_t[t]), int(Mhi_t[t])
            for (ri, cli, nni, M, ia, off) in (
                (rl, cll, nl, Mlo, ilo_a, lo_off),
                (rh, clh, nh, Mhi, ihi_a, hi_off),
            ):
                if M == 0:
                    continue
                n_e = len(ri)
                pi = np.zeros(M * 128, dtype=np.int64)
                pc = np.full(M * 128, -1.0, dtype=np.float32)
                pn = np.zeros(M * 128, dtype=np.float32)
                pi[:n_e] = ri
                pc[:n_e] = cli
                pn[:n_e] = nni
                ia[:, off * 8 : (off + M) * 8] = _wrap16(pi)
                # chunk j, partition p <- list position j*128+p
                cols_a[:, ci : ci + M] = pc.reshape(M, 128).T
                nrm_a[:, ci : ci + M] = pn.reshape(M, 128).T
                ci += M
            lo_off += Mlo
            hi_off += Mhi

        x0t = np.zeros((D, NPAD), dtype=np.float32)
        x0t[:, :NPC] = (ALPHA * x0[c * NPC : (c + 1) * NPC]).T

        in_maps.append(
            {
                "xlo": xlo,
                "xhi": xhi,
                "ilo": ilo_a,
                "ihi": ihi_a,
                "cols": cols_a,
                "nrm": nrm_a,
                "iot": iot,
                "x0t": np.ascontiguousarray(x0t),
                "wl": wl,
            }
        )
    return schedule, in_maps


def kernel(x, x0, edge_index, norm, W):
    global LAST
    from concourse.bass_utils import run_bass_kernel_spmd

    schedule, in_maps = _preprocess(x, x0, edge_index, norm, W)
    key = tuple(schedule)
    if key not in _prog_cache:
        _prog_cache[key] = _build_program(schedule)
    nc = _prog_cache[key]

    trace = os.environ.get("KERNEL_TRACE", "0") == "1"
    res = run_bass_kernel_spmd(
        nc,
        in_maps,
        core_ids=list(range(NCORES)),
        trace=trace,
    )
    LAST = res

    y = np.empty((N, D), dtype=np.float32)
    for c in range(NCORES):
        y[c * NPC : (c + 1) * NPC] = res.results[c]["yt"][:, :NPC].T
    return y


# revision 5
# speedup vs baseline: 1.0678x; 1.0678x over previous
"""GCNII conv kernel for 8 Trainium2 NeuronCores.

Strategy (self-contained; shapes hardcoded):
  - Shard destination nodes across 8 cores (6250 each); edges partitioned by
    destination so each core's segment_sum is local.
  - Host pre-pass: sort edges by dest, group into 128-dest tiles, split each
    tile's edges by source half (int16 gather indices), pad each half to a
    multiple of 128 ("chunks"); per-tile chunk counts are the max over cores
    so all cores run one identical program.
  - Device, per dest tile:
      * dma_gather pulls all the tile's source rows x[row] (512B each) into
        SBUF as [128 edges, chunk, 128 feat] (idx i -> dst[i%128, i//128, :])
      * per chunk, one fused DVE op builds the scaled scatter matrix
        S[e, d] = 0.9*norm[e] * (col_local[e] == d)   (iota==col, then *norm)
      * PE accumulates segT[f, d] += msgs[e, f].T @ S[e, d] in PSUM
      * hT = segT + (0.1*x0).T tile  (alpha folded on host)
      * yT = W_eff @ hT via one matmul, W_eff = (1-beta)*I + beta*W folded on
        host, so no extra elementwise work
  - Output is produced transposed ([128, n_local]) and flipped back on host.
"""

import os
import sys

sys.path.insert(0, "/opt/trn_rl_repo")

import numpy as np

N = 50000
D = 128
NCORES = 8
NPC = N // NCORES          # 6250 dest nodes per core
TPC = (NPC + 127) // 128   # 49 dest tiles per core
NPAD = TPC * 128           # 6272
HALF = N // 2              # int16 gather index split
ALPHA = 0.1
THETA = 0.5
LAYER = 1

_prog_cache = {}

# Stash of the last BassKernelResults for test.py to inspect (exec_time_ns).
LAST = None


def _build_program(schedule):
    """schedule: list of (Mlo, Mhi) per dest tile (shared across cores)."""
    import concourse.bacc as bacc
    import concourse.mybir as mybir
    import concourse.tile as tile
    from concourse import library_config

    f32 = mybir.dt.float32
    bf16 = mybir.dt.bfloat16
    i16 = mybir.dt.int16
    TC = sum(ml + mh for ml, mh in schedule)
    CLO8 = sum(ml for ml, _ in schedule) * 8
    CHI8 = sum(mh for _, mh in schedule) * 8

    nc = bacc.Bacc(
        "TRN2", target_bir_lowering=False, debug=False, num_devices=NCORES
    )
    xlo = nc.dram_tensor("xlo", [HALF, D], bf16, kind="ExternalInput").ap()
    xhi = nc.dram_tensor("xhi", [N - HALF, D], bf16, kind="ExternalInput").ap()
    ilo = nc.dram_tensor("ilo", [128, CLO8], i16, kind="ExternalInput").ap()
    ihi = nc.dram_tensor("ihi", [128, CHI8], i16, kind="ExternalInput").ap()
    cols = nc.dram_tensor("cols", [128, TC], f32, kind="ExternalInput").ap()
    nrm = nc.dram_tensor("nrm", [128, TC], f32, kind="ExternalInput").ap()
    iot = nc.dram_tensor("iot", [128, 128], f32, kind="ExternalInput").ap()
    x0t = nc.dram_tensor("x0t", [D, NPAD], f32, kind="ExternalInput").ap()
    wl = nc.dram_tensor("wl", [D, D], f32, kind="ExternalInput").ap()
    yt = nc.dram_tensor("yt", [D, NPAD], f32, kind="ExternalOutput").ap()

    with tile.TileContext(nc) as tc:
        with (
            tc.tile_pool(name="persist", bufs=1) as pp,
            tc.tile_pool(name="msgs", bufs=3) as mp,
            tc.tile_pool(name="sel", bufs=6) as sp,
            tc.tile_pool(name="hout", bufs=2) as hp,
            tc.tile_pool(name="io", bufs=2) as iop,
            tc.tile_pool(name="pseg", bufs=2, space="PSUM") as psp,
            tc.tile_pool(name="py", bufs=2, space="PSUM") as pyp,
        ):
            nc.gpsimd.load_library(library_config.mlp)

            ilo_sb = pp.tile([128, CLO8], i16)
            ihi_sb = pp.tile([128, CHI8], i16)
            cols_sb = pp.tile([128, TC], f32)
            nrm_sb = pp.tile([128, TC], f32)
            wl_sb = pp.tile([128, 128], f32)
            iota_f = pp.tile([128, 128], f32)

            nc.sync.dma_start(ilo_sb[:], ilo[:, :])
            nc.sync.dma_start(ihi_sb[:], ihi[:, :])
            nc.sync.dma_start(cols_sb[:], cols[:, :])
            nc.sync.dma_start(nrm_sb[:], nrm[:, :])
            nc.sync.dma_start(wl_sb[:], wl[:, :])
            nc.sync.dma_start(iota_f[:], iot[:, :])

            ci = 0
            lo_off = 0
            hi_off = 0
            for t, (Mlo, Mhi) in enumerate(schedule):
                M = Mlo + Mhi
                msgs = mp.tile([128, M, 128], bf16, tag="msgs")
                if Mlo:
                    nc.gpsimd.dma_gather(
                        msgs[:, 0:Mlo, :],
                        xlo[:, :],
                        ilo_sb[:, lo_off * 8 : (lo_off + Mlo) * 8],
                        Mlo * 128,
                        Mlo * 128,
                        D,
                    )
                if Mhi:
                    nc.gpsimd.dma_gather(
                        msgs[:, Mlo:M, :],
                        xhi[:, :],
                        ihi_sb[:, hi_off * 8 : (hi_off + Mhi) * 8],
                        Mhi * 128,
                        Mhi * 128,
                        D,
                    )
                ps = psp.tile([128, 128], f32, space="PSUM", tag="pseg")
                for j in range(M):
                    S = sp.tile([128, 128], bf16, tag="sel")
                    nc.vector.tensor_scalar(
                        out=S[:],
                        in0=iota_f[:],
                        scalar1=cols_sb[:, ci + j : ci + j + 1],
                        scalar2=nrm_sb[:, ci + j : ci + j + 1],
                        op0=mybir.AluOpType.is_equal,
                        op1=mybir.AluOpType.mult,
                    )
                    nc.tensor.matmul(
                        ps[:],
                        lhsT=msgs[:, j, :],
                        rhs=S[:],
                        start=(j == 0),
                        stop=(j == M - 1),
                    )
                x0tile = iop.tile([128, 128], f32, tag="x0")
                nc.sync.dma_start(x0tile[:], x0t[:, t * 128 : (t + 1) * 128])
                hT = hp.tile([128, 128], f32, tag="h")
                nc.vector.tensor_tensor(
                    out=hT[:], in0=ps[:], in1=x0tile[:], op=mybir.AluOpType.add
                )
                yp = pyp.tile([128, 128], f32, space="PSUM", tag="py")
                nc.tensor.matmul(
                    yp[:], lhsT=wl_sb[:], rhs=hT[:], start=True, stop=True
                )
                yo = iop.tile([128, 128], f32, tag="yo")
                nc.vector.tensor_copy(yo[:], yp[:])
                nc.sync.dma_start(yt[:, t * 128 : (t + 1) * 128], yo[:])
                ci += M
                lo_off += Mlo
                hi_off += Mhi

    nc.compile()
    return nc


def _wrap16(idx_list):
    """int16 idx list (len = M*128) -> [128, M*8] wrapped+replicated layout:
    idx i is read from partition i%16, free slot i//16; replicate x8."""
    w = idx_list.reshape(-1, 16).T.astype(np.int16)  # [16, M*8]
    return np.tile(w, (8, 1))


def _preprocess(x, x0, edge_index, norm, W):
    row = np.ascontiguousarray(edge_index[0]).astype(np.int64)
    col = np.ascontiguousarray(edge_index[1]).astype(np.int64)
    norm = np.ascontiguousarray(norm).astype(np.float32)
    x = np.ascontiguousarray(x).astype(np.float32)
    x0 = np.ascontiguousarray(x0).astype(np.float32)
    W = np.ascontiguousarray(W).astype(np.float32)

    order = np.argsort(col, kind="stable")
    rs = row[order]
    cs = col[order]
    ns = (1.0 - ALPHA) * norm[order]

    bounds = np.empty((NCORES, TPC + 1), dtype=np.int64)
    for c in range(NCORES):
        lo = c * NPC
        cuts = lo + np.minimum(np.arange(TPC + 1) * 128, NPC)
        bounds[c] = np.searchsorted(cs, cuts, side="left")

    # Per (core, tile): split by source half, count chunks per half.
    per_ct = {}
    Mlo_ct = np.zeros((NCORES, TPC), dtype=np.int64)
    Mhi_ct = np.zeros((NCORES, TPC), dtype=np.int64)
    for c in range(NCORES):
        for t in range(TPC):
            e0, e1 = bounds[c, t], bounds[c, t + 1]
            r = rs[e0:e1]
            cl = (cs[e0:e1] - (c * NPC + t * 128)).astype(np.float32)
            nn = ns[e0:e1]
            m = r < HALF
            per_ct[(c, t)] = (r[m], cl[m], nn[m], r[~m] - HALF, cl[~m], nn[~m])
            Mlo_ct[c, t] = -(-len(r[m]) // 128)
            Mhi_ct[c, t] = -(-(len(r) - len(r[m])) // 128)

    Mlo_t = Mlo_ct.max(axis=0)
    Mhi_t = Mhi_ct.max(axis=0)
    # Guarantee at least one chunk per tile so PSUM is always written.
    zero = (Mlo_t + Mhi_t) == 0
    Mlo_t[zero] = 1
    schedule = [(int(a), int(b)) for a, b in zip(Mlo_t, Mhi_t)]
    TC = int((Mlo_t + Mhi_t).sum())
    CLO = int(Mlo_t.sum())
    CHI = int(Mhi_t.sum())

    beta = np.float32(np.log(THETA / LAYER + 1.0))
    W_eff = (1.0 - beta) * np.eye(D, dtype=np.float32) + beta * W
    wl = np.ascontiguousarray(W_eff.T)
    import ml_dtypes

    bf = ml_dtypes.bfloat16
    iot = np.ascontiguousarray(
        np.tile(np.arange(128, dtype=np.float32)[None, :], (128, 1))
    )
    xlo = np.ascontiguousarray(x[:HALF]).astype(bf)
    xhi = np.ascontiguousarray(x[HALF:]).astype(bf)

    in_maps = []
    for c in range(NCORES):
        ilo_a = np.zeros((128, CLO * 8), dtype=np.int16)
        ihi_a = np.zeros((128, CHI * 8), dtype=np.int16)
        cols_a = np.full((128, TC), -1.0, dtype=np.float32)
        nrm_a = np.zeros((128, TC), dtype=np.float32)
        ci = 0
        lo_off = 0
        hi_off = 0
        for t in range(TPC):
            rl, cll, nl, rh, clh, nh = per_ct[(c, t)]
            Mlo, Mhi = int(Mlo_t[t]), int(Mhi_t[t])
            for (ri, cli, nni, M, ia, off) in (
                (rl, cll, nl, Mlo, ilo_a, lo_off),
                (rh, clh, nh, Mhi, ihi_a, hi_off),
            ):
                if M == 0:
                    continue
                n_e = len(ri)
                pi = np.zeros(M * 128, dtype=np.int64)
                pc = np.full(M * 128, -1.0, dtype=np.float32)
                pn = np.zeros(M * 128, dtype=np.float32)
                pi[:n_e] = ri
                pc[:n_e] = cli
                pn[:n_e] = nni
                ia[:, off * 8 : (off + M) * 8] = _wrap16(pi)
                # chunk j, partition p <- list position j*128+p
                cols_a[:, ci : ci + M] = pc.reshape(M, 128).T
                nrm_a[:, ci : ci + M] = pn.reshape(M, 128).T
                ci += M
            lo_off += Mlo
            hi_off += Mhi

        x0t = np.zeros((D, NPAD), dtype=np.float32)
        x0t[:, :NPC] = (ALPHA * x0[c * NPC : (c + 1) * NPC]).T

        in_maps.append(
            {
                "xlo": xlo,
                "xhi": xhi,
                "ilo": ilo_a,
                "ihi": ihi_a,
                "cols": cols_a,
                "nrm": nrm_a,
                "iot": iot,
                "x0t": np.ascontiguousarray(x0t),
                "wl": wl,
            }
        )
    return schedule, in_maps


def kernel(x, x0, edge_index, norm, W):
    global LAST
    from concourse.bass_utils import run_bass_kernel_spmd

    schedule, in_maps = _preprocess(x, x0, edge_index, norm, W)
    key = tuple(schedule)
    if key not in _prog_cache:
        _prog_cache[key] = _build_program(schedule)
    nc = _prog_cache[key]

    trace = os.environ.get("KERNEL_TRACE", "0") == "1"
    res = run_bass_kernel_spmd(
        nc,
        in_maps,
        core_ids=list(range(NCORES)),
        trace=trace,
    )
    LAST = res

    y = np.empty((N, D), dtype=np.float32)
    for c in range(NCORES):
        y[c * NPC : (c + 1) * NPC] = res.results[c]["yt"][:, :NPC].T
    return y


# revision 6
# speedup vs baseline: 1.0827x; 1.0139x over previous
"""GCNII conv kernel for 8 Trainium2 NeuronCores.

Strategy (self-contained; shapes hardcoded):
  - Shard destination nodes across 8 cores (6250 each); edges partitioned by
    destination so each core's segment_sum is local.
  - Host pre-pass: sort edges by dest, group into 128-dest tiles, split each
    tile's edges by source half (int16 gather indices), pad each half to a
    multiple of 128 ("chunks"); per-tile chunk counts are the max over cores
    so all cores run one identical program.
  - Device, per dest tile:
      * dma_gather pulls all the tile's source rows x[row] (512B each) into
        SBUF as [128 edges, chunk, 128 feat] (idx i -> dst[i%128, i//128, :])
      * per chunk, one fused DVE op builds the scaled scatter matrix
        S[e, d] = 0.9*norm[e] * (col_local[e] == d)   (iota==col, then *norm)
      * PE accumulates segT[f, d] += msgs[e, f].T @ S[e, d] in PSUM
      * hT = segT + (0.1*x0).T tile  (alpha folded on host)
      * yT = W_eff @ hT via one matmul, W_eff = (1-beta)*I + beta*W folded on
        host, so no extra elementwise work
  - Output is produced transposed ([128, n_local]) and flipped back on host.
"""

import os
import sys

sys.path.insert(0, "/opt/trn_rl_repo")

import numpy as np

N = 50000
D = 128
NCORES = 8
NPC = N // NCORES          # 6250 dest nodes per core
TPC = (NPC + 127) // 128   # 49 dest tiles per core
NPAD = TPC * 128           # 6272
HALF = N // 2              # int16 gather index split
ALPHA = 0.1
THETA = 0.5
LAYER = 1

_prog_cache = {}

# Stash of the last BassKernelResults for test.py to inspect (exec_time_ns).
LAST = None


def _build_program(schedule):
    """schedule: list of (Mlo, Mhi) per dest tile (shared across cores)."""
    import concourse.bacc as bacc
    import concourse.mybir as mybir
    import concourse.tile as tile
    from concourse import library_config

    f32 = mybir.dt.float32
    bf16 = mybir.dt.bfloat16
    i16 = mybir.dt.int16
    TC = sum(ml + mh for ml, mh in schedule)
    CLO8 = sum(ml for ml, _ in schedule) * 8
    CHI8 = sum(mh for _, mh in schedule) * 8

    nc = bacc.Bacc(
        "TRN2", target_bir_lowering=False, debug=False, num_devices=NCORES
    )
    xlo = nc.dram_tensor("xlo", [HALF, D], bf16, kind="ExternalInput").ap()
    xhi = nc.dram_tensor("xhi", [N - HALF, D], bf16, kind="ExternalInput").ap()
    ilo = nc.dram_tensor("ilo", [128, CLO8], i16, kind="ExternalInput").ap()
    ihi = nc.dram_tensor("ihi", [128, CHI8], i16, kind="ExternalInput").ap()
    cols = nc.dram_tensor("cols", [128, TC], f32, kind="ExternalInput").ap()
    nrm = nc.dram_tensor("nrm", [128, TC], f32, kind="ExternalInput").ap()
    iot = nc.dram_tensor("iot", [128, 128], f32, kind="ExternalInput").ap()
    x0t = nc.dram_tensor("x0t", [D, NPAD], f32, kind="ExternalInput").ap()
    wl = nc.dram_tensor("wl", [D, D], f32, kind="ExternalInput").ap()
    yt = nc.dram_tensor("yt", [D, NPAD], f32, kind="ExternalOutput").ap()

    with tile.TileContext(nc) as tc:
        with (
            tc.tile_pool(name="persist", bufs=1) as pp,
            tc.tile_pool(name="msgs", bufs=3) as mp,
            tc.tile_pool(name="sel", bufs=6) as sp,
            tc.tile_pool(name="hout", bufs=2) as hp,
            tc.tile_pool(name="io", bufs=2) as iop,
            tc.tile_pool(name="pseg", bufs=2, space="PSUM") as psp,
            tc.tile_pool(name="py", bufs=2, space="PSUM") as pyp,
        ):
            nc.gpsimd.load_library(library_config.mlp)

            ilo_sb = pp.tile([128, CLO8], i16)
            ihi_sb = pp.tile([128, CHI8], i16)
            cols_sb = pp.tile([128, TC], f32)
            nrm_sb = pp.tile([128, TC], f32)
            wl_sb = pp.tile([128, 128], f32)
            iota_f = pp.tile([128, 128], f32)

            nc.sync.dma_start(ilo_sb[:], ilo[:, :])
            nc.sync.dma_start(ihi_sb[:], ihi[:, :])
            nc.sync.dma_start(cols_sb[:], cols[:, :])
            nc.sync.dma_start(nrm_sb[:], nrm[:, :])
            nc.sync.dma_start(wl_sb[:], wl[:, :])
            nc.sync.dma_start(iota_f[:], iot[:, :])

            ci = 0
            lo_off = 0
            hi_off = 0
            for t, (Mlo, Mhi) in enumerate(schedule):
                M = Mlo + Mhi
                msgs = mp.tile([128, M, 128], bf16, tag="msgs")
                if Mlo:
                    nc.gpsimd.dma_gather(
                        msgs[:, 0:Mlo, :],
                        xlo[:, :],
                        ilo_sb[:, lo_off * 8 : (lo_off + Mlo) * 8],
                        Mlo * 128,
                        Mlo * 128,
                        D,
                    )
                if Mhi:
                    nc.gpsimd.dma_gather(
                        msgs[:, Mlo:M, :],
                        xhi[:, :],
                        ihi_sb[:, hi_off * 8 : (hi_off + Mhi) * 8],
                        Mhi * 128,
                        Mhi * 128,
                        D,
                    )
                ps = psp.tile([128, 128], f32, space="PSUM", tag="pseg")
                for j in range(M):
                    S = sp.tile([128, 128], bf16, tag="sel")
                    nc.vector.tensor_scalar(
                        out=S[:],
                        in0=iota_f[:],
                        scalar1=cols_sb[:, ci + j : ci + j + 1],
                        scalar2=nrm_sb[:, ci + j : ci + j + 1],
                        op0=mybir.AluOpType.is_equal,
                        op1=mybir.AluOpType.mult,
                    )
                    nc.tensor.matmul(
                        ps[:],
                        lhsT=msgs[:, j, :],
                        rhs=S[:],
                        start=(j == 0),
                        stop=(j == M - 1),
                    )
                x0tile = iop.tile([128, 128], f32, tag="x0")
                nc.sync.dma_start(x0tile[:], x0t[:, t * 128 : (t + 1) * 128])
                hT = hp.tile([128, 128], f32, tag="h")
                nc.vector.tensor_tensor(
                    out=hT[:], in0=ps[:], in1=x0tile[:], op=mybir.AluOpType.add
                )
                yp = pyp.tile([128, 128], f32, space="PSUM", tag="py")
                nc.tensor.matmul(
                    yp[:], lhsT=wl_sb[:], rhs=hT[:], start=True, stop=True
                )
                yo = iop.tile([128, 128], f32, tag="yo")
                nc.vector.tensor_copy(yo[:], yp[:])
                nc.sync.dma_start(yt[:, t * 128 : (t + 1) * 128], yo[:])
                ci += M
                lo_off += Mlo
                hi_off += Mhi

    nc.compile()
    return nc


def _wrap16(idx_list):
    """int16 idx list (len = M*128) -> [128, M*8] wrapped+replicated layout:
    idx i is read from partition i%16, free slot i//16; replicate x8."""
    w = idx_list.reshape(-1, 16).T.astype(np.int16)  # [16, M*8]
    return np.tile(w, (8, 1))


def _preprocess(x, x0, edge_index, norm, W):
    row = np.ascontiguousarray(edge_index[0]).astype(np.int64)
    col = np.ascontiguousarray(edge_index[1]).astype(np.int64)
    norm = np.ascontiguousarray(norm).astype(np.float32)
    x = np.ascontiguousarray(x).astype(np.float32)
    x0 = np.ascontiguousarray(x0).astype(np.float32)
    W = np.ascontiguousarray(W).astype(np.float32)

    order = np.argsort(col, kind="stable")
    rs = row[order]
    cs = col[order]
    ns = (1.0 - ALPHA) * norm[order]

    # Global 128-dest tiles, snake-dealt to cores by edge count so per-slot
    # chunk counts are balanced (minimizes shared-schedule padding).
    NT = (N + 127) // 128  # 391
    tstart = np.arange(NT) * 128
    tend = np.minimum(tstart + 128, N)
    e_lo = np.searchsorted(cs, tstart, side="left")
    e_hi = np.searchsorted(cs, tend, side="left")
    cnt = e_hi - e_lo

    order_t = np.argsort(-cnt, kind="stable")
    SLOTS = TPC  # 49 rounds
    assign = -np.ones((NCORES, SLOTS), dtype=np.int64)  # -1 = dummy tile
    k = 0
    for r in range(SLOTS):
        picks = order_t[k : k + NCORES]
        k += len(picks)
        cores = range(NCORES) if r % 2 == 0 else range(NCORES - 1, -1, -1)
        for i, c in enumerate(cores):
            if i < len(picks):
                assign[c, r] = picks[i]

    # Per (core, slot): lo/hi edge lists
    per_ct = {}
    Mlo_ct = np.zeros((NCORES, SLOTS), dtype=np.int64)
    Mhi_ct = np.zeros((NCORES, SLOTS), dtype=np.int64)
    for c in range(NCORES):
        for t in range(SLOTS):
            g = assign[c, t]
            if g < 0:
                per_ct[(c, t)] = None
                continue
            e0, e1 = e_lo[g], e_hi[g]
            r = rs[e0:e1]
            cl = (cs[e0:e1] - tstart[g]).astype(np.float32)
            nn2 = ns[e0:e1]
            m = r < HALF
            per_ct[(c, t)] = (r[m], cl[m], nn2[m], r[~m] - HALF, cl[~m], nn2[~m])
            Mlo_ct[c, t] = -(-int(m.sum()) // 128)
            Mhi_ct[c, t] = -(-int((~m).sum()) // 128)

    Mlo_t = Mlo_ct.max(axis=0)
    Mhi_t = Mhi_ct.max(axis=0)
    zero = (Mlo_t + Mhi_t) == 0
    Mlo_t[zero] = 1
    schedule = [(int(a), int(b)) for a, b in zip(Mlo_t, Mhi_t)]
    TC = int((Mlo_t + Mhi_t).sum())
    CLO = int(Mlo_t.sum())
    CHI = int(Mhi_t.sum())

    beta = np.float32(np.log(THETA / LAYER + 1.0))
    W_eff = (1.0 - beta) * np.eye(D, dtype=np.float32) + beta * W
    wl = np.ascontiguousarray(W_eff.T)
    import ml_dtypes

    bf = ml_dtypes.bfloat16
    iot = np.ascontiguousarray(
        np.tile(np.arange(128, dtype=np.float32)[None, :], (128, 1))
    )
    xlo = np.ascontiguousarray(x[:HALF]).astype(bf)
    xhi = np.ascontiguousarray(x[HALF:]).astype(bf)

    in_maps = []
    for c in range(NCORES):
        ilo_a = np.zeros((128, CLO * 8), dtype=np.int16)
        ihi_a = np.zeros((128, CHI * 8), dtype=np.int16)
        cols_a = np.full((128, TC), -1.0, dtype=np.float32)
        nrm_a = np.zeros((128, TC), dtype=np.float32)
        x0t = np.zeros((D, NPAD), dtype=np.float32)
        ci = 0
        lo_off = 0
        hi_off = 0
        for t in range(SLOTS):
            Mlo, Mhi = int(Mlo_t[t]), int(Mhi_t[t])
            data = per_ct[(c, t)]
            if data is not None:
                rl, cll, nl, rh, clh, nh = data
                g = assign[c, t]
                sz = int(tend[g] - tstart[g])
                x0t[:, t * 128 : t * 128 + sz] = (
                    ALPHA * x0[tstart[g] : tend[g]]
                ).T
            else:
                rl = cll = nl = rh = clh = nh = np.zeros(0)
            for (ri, cli, nni, M, ia, off) in (
                (rl, cll, nl, Mlo, ilo_a, lo_off),
                (rh, clh, nh, Mhi, ihi_a, hi_off),
            ):
                if M == 0:
                    continue
                n_e = len(ri)
                pi = np.zeros(M * 128, dtype=np.int64)
                pc = np.full(M * 128, -1.0, dtype=np.float32)
                pn = np.zeros(M * 128, dtype=np.float32)
                pi[:n_e] = ri
                pc[:n_e] = cli
                pn[:n_e] = nni
                ia[:, off * 8 : (off + M) * 8] = _wrap16(pi)
                cols_a[:, ci : ci + M] = pc.reshape(M, 128).T
                nrm_a[:, ci : ci + M] = pn.reshape(M, 128).T
                ci += M
            lo_off += Mlo
            hi_off += Mhi

        in_maps.append(
            {
                "xlo": xlo,
                "xhi": xhi,
                "ilo": ilo_a,
                "ihi": ihi_a,
                "cols": cols_a,
                "nrm": nrm_a,
                "iot": iot,
                "x0t": np.ascontiguousarray(x0t),
                "wl": wl,
            }
        )
    return schedule, in_maps, (assign, tstart, tend)


def kernel(x, x0, edge_index, norm, W):
    global LAST
    from concourse.bass_utils import run_bass_kernel_spmd

    schedule, in_maps, (assign, tstart, tend) = _preprocess(
        x, x0, edge_index, norm, W
    )
    key = tuple(schedule)
    if key not in _prog_cache:
        _prog_cache[key] = _build_program(schedule)
    nc = _prog_cache[key]

    trace = os.environ.get("KERNEL_TRACE", "0") == "1"
    res = run_bass_kernel_spmd(
        nc,
        in_maps,
        core_ids=list(range(NCORES)),
        trace=trace,
    )
    LAST = res

    y = np.empty((N, D), dtype=np.float32)
    for c in range(NCORES):
        yt = res.results[c]["yt"]
        for t in range(TPC):
            g = assign[c, t]
            if g < 0:
                continue
            sz = int(tend[g] - tstart[g])
            y[tstart[g] : tend[g]] = yt[:, t * 128 : t * 128 + sz].T
    return y


# revision 7
# speedup vs baseline: 1.1008x; 1.0167x over previous
"""GCNII conv kernel for 8 Trainium2 NeuronCores.

Strategy (self-contained; shapes hardcoded):
  - Shard destination nodes across 8 cores (6250 each); edges partitioned by
    destination so each core's segment_sum is local.
  - Host pre-pass: sort edges by dest, group into 128-dest tiles, split each
    tile's edges by source half (int16 gather indices), pad each half to a
    multiple of 128 ("chunks"); per-tile chunk counts are the max over cores
    so all cores run one identical program.
  - Device, per dest tile:
      * dma_gather pulls all the tile's source rows x[row] (512B each) into
        SBUF as [128 edges, chunk, 128 feat] (idx i -> dst[i%128, i//128, :])
      * per chunk, one fused DVE op builds the scaled scatter matrix
        S[e, d] = 0.9*norm[e] * (col_local[e] == d)   (iota==col, then *norm)
      * PE accumulates segT[f, d] += msgs[e, f].T @ S[e, d] in PSUM
      * hT = segT + (0.1*x0).T tile  (alpha folded on host)
      * yT = W_eff @ hT via one matmul, W_eff = (1-beta)*I + beta*W folded on
        host, so no extra elementwise work
  - Output is produced transposed ([128, n_local]) and flipped back on host.
"""

import os
import sys

sys.path.insert(0, "/opt/trn_rl_repo")

import numpy as np

N = 50000
D = 128
NCORES = 8
NPC = N // NCORES          # 6250 dest nodes per core
TPC = (NPC + 127) // 128   # 49 dest tiles per core
NPAD = TPC * 128           # 6272
HALF = N // 2              # int16 gather index split
ALPHA = 0.1
THETA = 0.5
LAYER = 1

_prog_cache = {}

# Stash of the last BassKernelResults for test.py to inspect (exec_time_ns).
LAST = None


def _build_program(schedule):
    """schedule: list of (Mlo, Mhi) per dest tile (shared across cores)."""
    import concourse.bacc as bacc
    import concourse.mybir as mybir
    import concourse.tile as tile
    from concourse import library_config

    f32 = mybir.dt.float32
    bf16 = mybir.dt.bfloat16
    i16 = mybir.dt.int16
    TC = sum(ml + mh for ml, mh in schedule)
    CLO8 = sum(ml for ml, _ in schedule) * 8
    CHI8 = sum(mh for _, mh in schedule) * 8

    nc = bacc.Bacc(
        "TRN2", target_bir_lowering=False, debug=False, num_devices=NCORES
    )
    xlo = nc.dram_tensor("xlo", [HALF, D], bf16, kind="ExternalInput").ap()
    xhi = nc.dram_tensor("xhi", [N - HALF, D], bf16, kind="ExternalInput").ap()
    ilo = nc.dram_tensor("ilo", [128, CLO8], i16, kind="ExternalInput").ap()
    ihi = nc.dram_tensor("ihi", [128, CHI8], i16, kind="ExternalInput").ap()
    cols = nc.dram_tensor("cols", [128, TC], f32, kind="ExternalInput").ap()
    nrm = nc.dram_tensor("nrm", [128, TC], f32, kind="ExternalInput").ap()
    iot = nc.dram_tensor("iot", [128, 128], f32, kind="ExternalInput").ap()
    x0t = nc.dram_tensor("x0t", [D, NPAD], f32, kind="ExternalInput").ap()
    wl = nc.dram_tensor("wl", [D, D], f32, kind="ExternalInput").ap()
    yt = nc.dram_tensor("yt", [D, NPAD], f32, kind="ExternalOutput").ap()

    with tile.TileContext(nc) as tc:
        with (
            tc.tile_pool(name="persist", bufs=1) as pp,
            tc.tile_pool(name="msgs", bufs=3) as mp,
            tc.tile_pool(name="sel", bufs=6) as sp,
            tc.tile_pool(name="hout", bufs=2) as hp,
            tc.tile_pool(name="io", bufs=2) as iop,
            tc.tile_pool(name="pseg", bufs=2, space="PSUM") as psp,
            tc.tile_pool(name="py", bufs=2, space="PSUM") as pyp,
        ):
            nc.gpsimd.load_library(library_config.mlp)

            ilo_sb = pp.tile([128, CLO8], i16)
            ihi_sb = pp.tile([128, CHI8], i16)
            cols_sb = pp.tile([128, TC], f32)
            nrm_sb = pp.tile([128, TC], f32)
            wl_sb = pp.tile([128, 128], f32)
            iota_f = pp.tile([128, 128], f32)

            nc.sync.dma_start(ilo_sb[:], ilo[:, :])
            nc.sync.dma_start(ihi_sb[:], ihi[:, :])
            nc.sync.dma_start(cols_sb[:], cols[:, :])
            nc.sync.dma_start(nrm_sb[:], nrm[:, :])
            nc.sync.dma_start(wl_sb[:], wl[:, :])
            nc.sync.dma_start(iota_f[:], iot[:, :])

            ci = 0
            lo_off = 0
            hi_off = 0
            for t, (Mlo, Mhi) in enumerate(schedule):
                M = Mlo + Mhi
                msgs = mp.tile([128, M, 128], bf16, tag="msgs")
                if Mlo:
                    nc.gpsimd.dma_gather(
                        msgs[:, 0:Mlo, :],
                        xlo[:, :],
                        ilo_sb[:, lo_off * 8 : (lo_off + Mlo) * 8],
                        Mlo * 128,
                        Mlo * 128,
                        D,
                        single_packet=False,
                    )
                if Mhi:
                    nc.gpsimd.dma_gather(
                        msgs[:, Mlo:M, :],
                        xhi[:, :],
                        ihi_sb[:, hi_off * 8 : (hi_off + Mhi) * 8],
                        Mhi * 128,
                        Mhi * 128,
                        D,
                        single_packet=False,
                    )
                ps = psp.tile([128, 128], f32, space="PSUM", tag="pseg")
                for j in range(M):
                    S = sp.tile([128, 128], bf16, tag="sel")
                    nc.vector.tensor_scalar(
                        out=S[:],
                        in0=iota_f[:],
                        scalar1=cols_sb[:, ci + j : ci + j + 1],
                        scalar2=nrm_sb[:, ci + j : ci + j + 1],
                        op0=mybir.AluOpType.is_equal,
                        op1=mybir.AluOpType.mult,
                    )
                    nc.tensor.matmul(
                        ps[:],
                        lhsT=msgs[:, j, :],
                        rhs=S[:],
                        start=(j == 0),
                        stop=(j == M - 1),
                    )
                x0tile = iop.tile([128, 128], f32, tag="x0")
                nc.sync.dma_start(x0tile[:], x0t[:, t * 128 : (t + 1) * 128])
                hT = hp.tile([128, 128], f32, tag="h")
                nc.vector.tensor_tensor(
                    out=hT[:], in0=ps[:], in1=x0tile[:], op=mybir.AluOpType.add
                )
                yp = pyp.tile([128, 128], f32, space="PSUM", tag="py")
                nc.tensor.matmul(
                    yp[:], lhsT=wl_sb[:], rhs=hT[:], start=True, stop=True
                )
                yo = iop.tile([128, 128], f32, tag="yo")
                nc.vector.tensor_copy(yo[:], yp[:])
                nc.sync.dma_start(yt[:, t * 128 : (t + 1) * 128], yo[:])
                ci += M
                lo_off += Mlo
                hi_off += Mhi

    nc.compile()
    return nc


def _wrap16(idx_list):
    """int16 idx list (len = M*128) -> [128, M*8] wrapped+replicated layout:
    idx i is read from partition i%16, free slot i//16; replicate x8."""
    w = idx_list.reshape(-1, 16).T.astype(np.int16)  # [16, M*8]
    return np.tile(w, (8, 1))


def _preprocess(x, x0, edge_index, norm, W):
    row = np.ascontiguousarray(edge_index[0]).astype(np.int64)
    col = np.ascontiguousarray(edge_index[1]).astype(np.int64)
    norm = np.ascontiguousarray(norm).astype(np.float32)
    x = np.ascontiguousarray(x).astype(np.float32)
    x0 = np.ascontiguousarray(x0).astype(np.float32)
    W = np.ascontiguousarray(W).astype(np.float32)

    order = np.argsort(col, kind="stable")
    rs = row[order]
    cs = col[order]
    ns = (1.0 - ALPHA) * norm[order]

    # Global 128-dest tiles, snake-dealt to cores by edge count so per-slot
    # chunk counts are balanced (minimizes shared-schedule padding).
    NT = (N + 127) // 128  # 391
    tstart = np.arange(NT) * 128
    tend = np.minimum(tstart + 128, N)
    e_lo = np.searchsorted(cs, tstart, side="left")
    e_hi = np.searchsorted(cs, tend, side="left")
    cnt = e_hi - e_lo

    order_t = np.argsort(-cnt, kind="stable")
    SLOTS = TPC  # 49 rounds
    assign = -np.ones((NCORES, SLOTS), dtype=np.int64)  # -1 = dummy tile
    k = 0
    for r in range(SLOTS):
        picks = order_t[k : k + NCORES]
        k += len(picks)
        cores = range(NCORES) if r % 2 == 0 else range(NCORES - 1, -1, -1)
        for i, c in enumerate(cores):
            if i < len(picks):
                assign[c, r] = picks[i]

    # Per (core, slot): lo/hi edge lists
    per_ct = {}
    Mlo_ct = np.zeros((NCORES, SLOTS), dtype=np.int64)
    Mhi_ct = np.zeros((NCORES, SLOTS), dtype=np.int64)
    for c in range(NCORES):
        for t in range(SLOTS):
            g = assign[c, t]
            if g < 0:
                per_ct[(c, t)] = None
                continue
            e0, e1 = e_lo[g], e_hi[g]
            r = rs[e0:e1]
            cl = (cs[e0:e1] - tstart[g]).astype(np.float32)
            nn2 = ns[e0:e1]
            m = r < HALF
            per_ct[(c, t)] = (r[m], cl[m], nn2[m], r[~m] - HALF, cl[~m], nn2[~m])
            Mlo_ct[c, t] = -(-int(m.sum()) // 128)
            Mhi_ct[c, t] = -(-int((~m).sum()) // 128)

    Mlo_t = Mlo_ct.max(axis=0)
    Mhi_t = Mhi_ct.max(axis=0)
    zero = (Mlo_t + Mhi_t) == 0
    Mlo_t[zero] = 1
    schedule = [(int(a), int(b)) for a, b in zip(Mlo_t, Mhi_t)]
    TC = int((Mlo_t + Mhi_t).sum())
    CLO = int(Mlo_t.sum())
    CHI = int(Mhi_t.sum())

    beta = np.float32(np.log(THETA / LAYER + 1.0))
    W_eff = (1.0 - beta) * np.eye(D, dtype=np.float32) + beta * W
    wl = np.ascontiguousarray(W_eff.T)
    import ml_dtypes

    bf = ml_dtypes.bfloat16
    iot = np.ascontiguousarray(
        np.tile(np.arange(128, dtype=np.float32)[None, :], (128, 1))
    )
    xlo = np.ascontiguousarray(x[:HALF]).astype(bf)
    xhi = np.ascontiguousarray(x[HALF:]).astype(bf)

    in_maps = []
    for c in range(NCORES):
        ilo_a = np.zeros((128, CLO * 8), dtype=np.int16)
        ihi_a = np.zeros((128, CHI * 8), dtype=np.int16)
        cols_a = np.full((128, TC), -1.0, dtype=np.float32)
        nrm_a = np.zeros((128, TC), dtype=np.float32)
        x0t = np.zeros((D, NPAD), dtype=np.float32)
        ci = 0
        lo_off = 0
        hi_off = 0
        for t in range(SLOTS):
            Mlo, Mhi = int(Mlo_t[t]), int(Mhi_t[t])
            data = per_ct[(c, t)]
            if data is not None:
                rl, cll, nl, rh, clh, nh = data
                g = assign[c, t]
                sz = int(tend[g] - tstart[g])
                x0t[:, t * 128 : t * 128 + sz] = (
                    ALPHA * x0[tstart[g] : tend[g]]
                ).T
            else:
                rl = cll = nl = rh = clh = nh = np.zeros(0)
            for (ri, cli, nni, M, ia, off) in (
                (rl, cll, nl, Mlo, ilo_a, lo_off),
                (rh, clh, nh, Mhi, ihi_a, hi_off),
            ):
                if M == 0:
                    continue
                n_e = len(ri)
                pi = np.zeros(M * 128, dtype=np.int64)
                pc = np.full(M * 128, -1.0, dtype=np.float32)
                pn = np.zeros(M * 128, dtype=np.float32)
                pi[:n_e] = ri
                pc[:n_e] = cli
                pn[:n_e] = nni
                ia[:, off * 8 : (off + M) * 8] = _wrap16(pi)
                cols_a[:, ci : ci + M] = pc.reshape(M, 128).T
                nrm_a[:, ci : ci + M] = pn.reshape(M, 128).T
                ci += M
            lo_off += Mlo
            hi_off += Mhi

        in_maps.append(
            {
                "xlo": xlo,
                "xhi": xhi,
                "ilo": ilo_a,
                "ihi": ihi_a,
                "cols": cols_a,
                "nrm": nrm_a,
                "iot": iot,
                "x0t": np.ascontiguousarray(x0t),
                "wl": wl,
            }
        )
    return schedule, in_maps, (assign, tstart, tend)


def kernel(x, x0, edge_index, norm, W):
    global LAST
    from concourse.bass_utils import run_bass_kernel_spmd

    schedule, in_maps, (assign, tstart, tend) = _preprocess(
        x, x0, edge_index, norm, W
    )
    key = tuple(schedule)
    if key not in _prog_cache:
        _prog_cache[key] = _build_program(schedule)
    nc = _prog_cache[key]

    trace = os.environ.get("KERNEL_TRACE", "0") == "1"
    res = run_bass_kernel_spmd(
        nc,
        in_maps,
        core_ids=list(range(NCORES)),
        trace=trace,
    )
    LAST = res

    y = np.empty((N, D), dtype=np.float32)
    for c in range(NCORES):
        yt = res.results[c]["yt"]
        for t in range(TPC):
            g = assign[c, t]
            if g < 0:
                continue
            sz = int(tend[g] - tstart[g])
            y[tstart[g] : tend[g]] = yt[:, t * 128 : t * 128 + sz].T
    return y
